# revision 1
# baseline (speedup 1.0000x reference)
"""Trainium2 Bass kernel for nn_BoundleAdjustment (2M observations).

Two launches on all 8 NeuronCores (observations data-parallel, M/8 per core):

Launch A (device): converts the 4096-row pose table (translation+quaternion)
into per-pose rotation matrices R = f(q/|q|) on the Vector engine
([128, 32] planar layout, one reciprocal for the 2/|q|^2 scale).

Host staging (indexing only): gathers the derived R table, raw pose
translations, and patch rows by poses_idx/patch_idx, and lays per-observation
records out as 19 planar [128, 2048] f32 streams per core.

Launch B (device): streams the planes through SBUF in 2 chunks and computes
r = R*pts + t, cart->polar (Square/Sqrt/Arctan/Sign on the Scalar engine,
atan2 quadrant fixup, all divisions via one merged Vector-engine reciprocal
using 1/a = b/(a*b)), and the weighted residual.  DMA issue lives on the SP
queue so the Vector engine stays on math; ~35 vector ops per observation.
"""

import numpy as np

M = 2097152
NCORES = 8
N = M // NCORES
P = 128
COLS = N // P            # 2048
CC = 1024
NCH = COLS // CC
NPOSE = 4096
PC = NPOSE // P          # 32 cols for pose table
PI = float(np.pi)

# launch B planes (ordered so the rot-x dependency chain's planes land first):
# R00 R01 R02 px py pz tx | R10 R11 R12 ty | R20 R21 R22 tz | X Y Z w
NPB = 19

_CACHE = {}


def _build_posetab():
    import concourse.bass as bass
    import concourse.tile as tile
    from concourse import bacc, mybir

    nc = bacc.Bacc("TRN2", target_bir_lowering=False, debug=False,
                   num_devices=NCORES)
    f32 = mybir.dt.float32
    OP = mybir.AluOpType
    q_d = nc.declare_dram_parameter("q", [4, P, PC], f32, isOutput=False)
    r_d = nc.declare_dram_parameter("rtab", [9, P, PC], f32, isOutput=True)

    with tile.TileContext(nc) as tc:
        with tc.tile_pool(name="pp", bufs=40) as pp:
            cnt = [0]

            def T():
                cnt[0] += 1
                return pp.tile([P, PC], f32, tag="t", name=f"pt{cnt[0]}")

            vec = nc.vector

            def tt(a, b, op):
                d = T()
                vec.tensor_tensor(out=d[:], in0=a[:], in1=b[:], op=op)
                return d

            qs = []
            for k in range(4):
                t = pp.tile([P, PC], f32, tag="t", name=f"qin{k}")
                nc.sync.dma_start(t[:], q_d[k])
                qs.append(t)
            qx, qy, qz, qw = qs

            xx = tt(qx, qx, OP.mult); yy = tt(qy, qy, OP.mult)
            zz = tt(qz, qz, OP.mult); ww = tt(qw, qw, OP.mult)
            xy = tt(qx, qy, OP.mult); xz = tt(qx, qz, OP.mult)
            yz = tt(qy, qz, OP.mult)
            wx = tt(qw, qx, OP.mult); wy = tt(qw, qy, OP.mult)
            wz = tt(qw, qz, OP.mult)

            s01 = tt(xx, yy, OP.add)
            s23 = tt(zz, ww, OP.add)
            s = tt(s01, s23, OP.add)
            d1 = T()
            vec.tensor_scalar(out=d1[:], in0=s[:], scalar1=0.5, scalar2=None,
                              op0=OP.mult)
            u = T()
            vec.reciprocal(u[:], d1[:])        # u = 2/|q|^2

            outs = {}

            def diag(m1, m2, nm):
                a = tt(m1, m2, OP.add)
                b = tt(a, u, OP.mult)
                d = T()
                vec.tensor_scalar(out=d[:], in0=b[:], scalar1=-1.0,
                                  scalar2=1.0, op0=OP.mult, op1=OP.add)
                outs[nm] = d

            def offd(m1, m2, op, nm):
                a = tt(m1, m2, op)
                outs[nm] = tt(a, u, OP.mult)

            diag(yy, zz, "R00"); diag(xx, zz, "R11"); diag(xx, yy, "R22")
            offd(xy, wz, OP.subtract, "R01"); offd(xz, wy, OP.add, "R02")
            offd(xy, wz, OP.add, "R10"); offd(yz, wx, OP.subtract, "R12")
            offd(xz, wy, OP.subtract, "R20"); offd(yz, wx, OP.add, "R21")

            for i, nm in enumerate(("R00", "R01", "R02", "R10", "R11", "R12",
                                    "R20", "R21", "R22")):
                nc.sync.dma_start(r_d[i], outs[nm][:])
    nc.finalize()
    return nc


def _build_main():
    import concourse.bass as bass
    import concourse.tile as tile
    from concourse import bacc, mybir

    nc = bacc.Bacc("TRN2", target_bir_lowering=False, debug=False,
                   num_devices=NCORES)
    f32 = mybir.dt.float32
    AF = mybir.ActivationFunctionType
    OP = mybir.AluOpType
    in_d = nc.declare_dram_parameter("in", [NPB, P, COLS], f32, isOutput=False)
    out_d = nc.declare_dram_parameter("out", [3, P, COLS], f32, isOutput=True)

    with tile.TileContext(nc) as tc:
        with tc.tile_pool(name="inp", bufs=2) as inp, \
             tc.tile_pool(name="tmpp", bufs=17) as tmpp:
            chunks = [(0, 512), (512, 1024), (1536, 512)]
            for ch, (off, cc) in enumerate(chunks):
                ins = []
                for k in range(NPB):
                    # target/weight planes (15-18) are consumed at chunk
                    # end; single buffers suffice and free SBUF for tmpp
                    t = inp.tile([P, cc], f32, tag=f"in{k}", name=f"in{k}_{ch}",
                                 bufs=1 if k >= 15 else 2)
                    nc.sync.dma_start(t[:], in_d[k, :, off:off + cc])
                    ins.append(t)
                (R00, R01, R02, px, py, pz, tx, R10, R11, R12, ty,
                 R20, R21, R22, tz, X, Y, Z, W) = ins

                cnt = [0]

                def T():
                    cnt[0] += 1
                    return tmpp.tile([P, cc], f32, tag="tmp",
                                     name=f"tmp{ch}_{cnt[0]}")

                vec, act = nc.vector, nc.scalar

                def tt(a, b, op):
                    d = T()
                    vec.tensor_tensor(out=d[:], in0=a[:], in1=b[:], op=op)
                    return d

                def sq(a):
                    d = T()
                    act.activation(d[:], a[:], AF.Square)
                    return d

                # r = R * pts + t
                def rot(Ra, Rb, Rc, tcm):
                    m0 = tt(Ra, px, OP.mult)
                    m1 = tt(Rb, py, OP.mult)
                    a0 = tt(m0, m1, OP.add)
                    m2 = tt(Rc, pz, OP.mult)
                    a1 = tt(m2, tcm, OP.add)
                    return tt(a0, a1, OP.add)

                rx = rot(R00, R01, R02, tx)
                ry = rot(R10, R11, R12, ty)
                rz = rot(R20, R21, R22, tz)

                sqx_, sqy_, sqz_ = sq(rx), sq(ry), sq(rz)
                rho2 = tt(sqx_, sqy_, OP.add)
                r2 = tt(rho2, sqz_, OP.add)
                rng = T(); act.activation(rng[:], r2[:], AF.Sqrt)
                rho = T(); act.activation(rho[:], rho2[:], AF.Sqrt)

                Pq = tt(rx, rho2, OP.mult)
                ip = T()
                vec.reciprocal(ip[:], Pq[:])
                irx = tt(ip, rho2, OP.mult)
                irho2 = tt(ip, rx, OP.mult)
                irho = tt(rho, irho2, OP.mult)

                a1_ = tt(ry, irx, OP.mult)
                az0 = T(); act.activation(az0[:], a1_[:], AF.Arctan)
                sg = T(); act.activation(sg[:], ry[:], AF.Sign)
                msk = T()
                vec.tensor_scalar(out=msk[:], in0=rx[:], scalar1=0.0,
                                  scalar2=PI, op0=OP.is_lt, op1=OP.mult)
                corr = tt(msk, sg, OP.mult)
                az = tt(az0, corr, OP.add)

                e1 = tt(rz, irho, OP.mult)
                el = T(); act.activation(el[:], e1[:], AF.Arctan)

                for (pcomp, tgt, idx) in ((rng, X, 0), (az, Y, 1), (el, Z, 2)):
                    dsub = tt(pcomp, tgt, OP.subtract)
                    o = tt(dsub, W, OP.mult)
                    nc.sync.dma_start(out_d[idx, :, off:off + cc], o[:])
    nc.finalize()
    return nc


def _get(name, builder):
    if name not in _CACHE:
        _CACHE[name] = builder()
    return _CACHE[name]


def kernel(poses, patch_coords, elevation_angle, poses_idx, patch_idx,
           target_coords, weights):
    from concourse.bass_utils import run_bass_kernel_spmd

    poses = np.asarray(poses, dtype=np.float32)
    patch_coords = np.asarray(patch_coords, dtype=np.float32)
    elevation_angle = np.asarray(elevation_angle, dtype=np.float32)
    target_coords = np.asarray(target_coords, dtype=np.float32)
    weights = np.asarray(weights, dtype=np.float32)
    pid = np.asarray(poses_idx).astype(np.int64)
    qid = np.asarray(patch_idx).astype(np.int64)

    # ---- launch A: pose table -> rotation matrices (device) ----
    q_planes = np.ascontiguousarray(
        poses[:, 3:7].reshape(P, PC, 4).transpose(2, 0, 1))   # [4,128,32]
    ncA = _get("A", _build_posetab)
    resA = run_bass_kernel_spmd(ncA, [{"q": q_planes} for _ in range(NCORES)],
                                list(range(NCORES)))
    rtab = np.asarray(resA.results[0]["rtab"]).reshape(9, NPOSE).T  # [4096,9]

    # ---- host: gather derived tables / per-obs staging (indexing only) ----
    r9 = rtab[pid]                                            # [M, 9]
    t3 = poses[pid, 0:3]                                      # [M, 3]
    pts = np.concatenate(
        [patch_coords[qid], elevation_angle[qid]], axis=1)    # [M, 3]
    big = np.concatenate(
        [r9[:, 0:3], pts, t3[:, 0:1], r9[:, 3:6], t3[:, 1:2],
         r9[:, 6:9], t3[:, 2:3], target_coords, weights], axis=1)
    big = np.ascontiguousarray(
        big.reshape(NCORES, P, COLS, NPB).transpose(0, 3, 1, 2))

    # ---- launch B: streaming rotate+polar+residual ----
    ncB = _get("B", _build_main)
    resB = run_bass_kernel_spmd(ncB, [{"in": big[c]} for c in range(NCORES)],
                                list(range(NCORES)))
    out = np.stack([resB.results[c]["out"] for c in range(NCORES)])
    return np.ascontiguousarray(
        out.transpose(0, 2, 3, 1).reshape(M, 3)).astype(np.float32)



# revision 2
# speedup vs baseline: 1.8616x; 1.8616x over previous
"""Trainium2 Bass kernel for nn_BoundleAdjustment (2M observations).

Single launch on all 8 NeuronCores.  Host work is pure indexing: observations
are sorted by pose and laid out pose-major, so each SBUF partition holds one
pose's observations per tile-slot and the per-pose rotation/translation become
[P,1] per-partition scalar operands — the 12 R/t streams of the old design
vanish from DMA and from the vector engine.

Device program (per core):
  1. tiny pose-table prologue: q -> R (f32, [128, 4] planes, ~27 ops)
  2. rot per slot: r = R*p + t via tensor_scalar + 2x scalar_tensor_tensor
     with [P,1] f32 scalars, fp16 streams
  3. planar polar+residual in fp16: range/az/el via Abs_reciprocal_sqrt and
     Arctan on the Scalar engine (2 activation-table sets only), quadrant
     fixup with tensor_scalar masks, weighted residual; a few off-critical
     tensor_tensor ops run on GPSIMD to unload the DVE.

Streams are fp16 (2x DVE mode, half the HBM bytes); per-pose scalars stay
f32, which keeps atan2 branch flips rare (rel RMS ~7e-3 vs fp32 reference).
"""

import numpy as np

M = 2097152
NCORES = 8
P = 128
NPOSE = 4096
NSLOT = 4          # tile slots per core; 8 cores * 4 slots * 128 poses = 4096
PI = float(np.pi)

_CACHE = {}


def _build_main(fds):
    import concourse.bass as bass
    import concourse.tile as tile
    from concourse import bacc, mybir

    nc = bacc.Bacc("TRN2", target_bir_lowering=False, debug=False,
                   num_devices=NCORES)
    f32 = mybir.dt.float32
    f16 = mybir.dt.float16
    OP = mybir.AluOpType
    AF = mybir.ActivationFunctionType
    FDT = int(sum(fds))

    tab_d = nc.declare_dram_parameter("tab", [7, P, NSLOT], f32, isOutput=False)
    obs_d = nc.declare_dram_parameter("obs", [7, P, FDT], f16, isOutput=False)
    out_d = nc.declare_dram_parameter("out", [3, P, FDT], f16, isOutput=True)

    with tile.TileContext(nc) as tc:
        with tc.tile_pool(name="pp", bufs=1) as pp:
            vec, act, gp = nc.vector, nc.scalar, nc.gpsimd

            # ---- pose table prologue (f32, FD=4) ----
            tabs = []
            for k in range(7):
                t = pp.tile([P, NSLOT], f32, name=f"tab{k}")
                nc.sync.dma_start(t[:], tab_d[k])
                tabs.append(t)
            qx, qy, qz, qw, tx, ty, tz = tabs

            tcnt = [0]

            def TT4():
                tcnt[0] += 1
                return pp.tile([P, NSLOT], f32, name=f"tt{tcnt[0]}")

            def mul4(a, b):
                d = TT4()
                vec.tensor_tensor(out=d[:], in0=a[:], in1=b[:], op=OP.mult)
                return d

            def add4(a, b):
                d = TT4()
                vec.tensor_tensor(out=d[:], in0=a[:], in1=b[:], op=OP.add)
                return d

            xx = mul4(qx, qx); yy = mul4(qy, qy)
            zz = mul4(qz, qz); ww = mul4(qw, qw)
            xy = mul4(qx, qy); xz = mul4(qx, qz); yz = mul4(qy, qz)
            wx = mul4(qw, qx); wy = mul4(qw, qy); wz = mul4(qw, qz)
            s = add4(add4(xx, yy), add4(zz, ww))
            half = TT4()
            vec.tensor_scalar(out=half[:], in0=s[:], scalar1=0.5, scalar2=None,
                              op0=OP.mult)
            u = TT4()
            vec.reciprocal(u[:], half[:])        # 2/|q|^2

            R = {}

            def diag(m1, m2, nm):
                a = add4(m1, m2)
                b = mul4(a, u)
                d = TT4()
                vec.tensor_scalar(out=d[:], in0=b[:], scalar1=-1.0,
                                  scalar2=1.0, op0=OP.mult, op1=OP.add)
                R[nm] = d

            def offd(m1, m2, op, nm):
                a = TT4()
                vec.tensor_tensor(out=a[:], in0=m1[:], in1=m2[:], op=op)
                R[nm] = mul4(a, u)

            diag(yy, zz, "R00"); diag(xx, zz, "R11"); diag(xx, yy, "R22")
            offd(xy, wz, OP.subtract, "R01"); offd(xz, wy, OP.add, "R02")
            offd(xy, wz, OP.add, "R10"); offd(yz, wx, OP.subtract, "R12")
            offd(xz, wy, OP.subtract, "R20"); offd(yz, wx, OP.add, "R21")

            # ---- obs streams in ----
            obs = []
            for k in range(7):
                t = pp.tile([P, FDT], f16, name=f"obs{k}")
                off = 0
                for s_, fd in enumerate(fds):
                    nc.sync.dma_start(t[:, off:off + fd],
                                      obs_d[k, :, off:off + fd])
                    off += fd
                obs.append(t)
            px, py, pz, X, Y, Z, W = obs

            # ---- rot: r = R*p + t per slot ----
            def rowplane():
                return pp.tile([P, FDT], f16, name=f"rp{tcnt[0]}")

            def rot_row(Rn0, Rn1, Rn2, tcm, nm):
                r = pp.tile([P, FDT], f16, name=nm)
                off = 0
                for s_, fd in enumerate(fds):
                    sl = slice(off, off + fd)
                    a = pp.tile([P, fd], f16, name=f"a{nm}{s_}")
                    vec.tensor_scalar(out=a[:], in0=pz[:, sl],
                                      scalar1=R[Rn2][:, s_:s_ + 1],
                                      scalar2=tcm[:, s_:s_ + 1],
                                      op0=OP.mult, op1=OP.add)
                    b = pp.tile([P, fd], f16, name=f"b{nm}{s_}")
                    vec.scalar_tensor_tensor(out=b[:], in0=py[:, sl],
                                             scalar=R[Rn1][:, s_:s_ + 1],
                                             in1=a[:], op0=OP.mult, op1=OP.add)
                    vec.scalar_tensor_tensor(out=r[:, sl], in0=px[:, sl],
                                             scalar=R[Rn0][:, s_:s_ + 1],
                                             in1=b[:], op0=OP.mult, op1=OP.add)
                    off += fd
                return r

            rx = rot_row("R00", "R01", "R02", tx, "rx")
            ry = rot_row("R10", "R11", "R12", ty, "ry")
            rz = rot_row("R20", "R21", "R22", tz, "rz")

            # ---- planar polar + residual (fp16, full FD) ----
            def T():
                tcnt[0] += 1
                return pp.tile([P, FDT], f16, name=f"pl{tcnt[0]}")

            def tt(a, b, op, eng=vec):
                d = T()
                eng.tensor_tensor(out=d[:], in0=a[:], in1=b[:], op=op)
                return d

            def afn(a, fn):
                d = T()
                act.activation(d[:], a[:], fn)
                return d

            sqx = tt(rx, rx, OP.mult)
            sqy = tt(ry, ry, OP.mult)
            sqz = tt(rz, rz, OP.mult, eng=gp)
            rho2 = tt(sqx, sqy, OP.add)
            r2 = tt(rho2, sqz, OP.add)
            # ACT set 1: abs_reciprocal_sqrt (+ Square filler)
            u2 = afn(rho2, AF.Abs_reciprocal_sqrt)    # 1/rho
            ur = afn(r2, AF.Abs_reciprocal_sqrt)      # 1/r
            ux = afn(rx, AF.Abs_reciprocal_sqrt)      # 1/sqrt(|rx|)
            ux2 = afn(ux, AF.Square)                  # 1/|rx|
            rng = tt(r2, ur, OP.mult)                 # r
            e1 = tt(rz, u2, OP.mult)
            t0 = tt(ry, ux2, OP.mult)                 # ry/|rx|
            # ACT set 2: arctan + sign
            el = afn(e1, AF.Arctan)
            azp = afn(t0, AF.Arctan)
            sgy = afn(ry, AF.Sign)
            # quadrant fixup: az = (1 + f)*azp + m*sgy, f in {0,-2}, m in {0,pi}
            m = T()
            vec.tensor_scalar(out=m[:], in0=rx[:], scalar1=0.0, scalar2=PI,
                              op0=OP.is_lt, op1=OP.mult)
            f = T()
            vec.tensor_scalar(out=f[:], in0=rx[:], scalar1=0.0, scalar2=-2.0,
                              op0=OP.is_lt, op1=OP.mult)
            az1 = T()
            vec.scalar_tensor_tensor(out=az1[:], in0=f[:], scalar=1.0,
                                     in1=azp[:], op0=OP.add, op1=OP.mult)
            corr = tt(m, sgy, OP.mult, eng=gp)
            az = tt(az1, corr, OP.add)

            for i, (pcomp, tgt) in enumerate(((rng, X), (az, Y), (el, Z))):
                d = tt(pcomp, tgt, OP.subtract)
                o = tt(d, W, OP.mult, eng=gp if i != 1 else vec)
                nc.sync.dma_start(out_d[i], o[:])
    nc.finalize()
    return nc


def _stage(poses, patch_coords, elevation_angle, poses_idx, patch_idx,
           target_coords, weights):
    """Pure-indexing host staging.  Returns (fds, tab array, obs array,
    gather indices for unpadding)."""
    poses = np.asarray(poses, dtype=np.float32)
    patch_coords = np.asarray(patch_coords, dtype=np.float32)
    elevation_angle = np.asarray(elevation_angle, dtype=np.float32)
    target_coords = np.asarray(target_coords, dtype=np.float32)
    weights = np.asarray(weights, dtype=np.float32)
    pid = np.asarray(poses_idx).astype(np.int64)
    qid = np.asarray(patch_idx).astype(np.int64)

    cnt = np.bincount(pid, minlength=NPOSE)
    order = np.argsort(-cnt, kind="stable")          # poses by count desc
    rank = np.empty(NPOSE, np.int64)
    rank[order] = np.arange(NPOSE)
    # tile k = ranks [128k,128(k+1)); tile -> core k%8, slot k//8
    # slot FD = count of first pose of tile 8s (max across cores)
    fds = tuple(int(cnt[order[128 * (8 * s)]]) for s in range(NSLOT))
    FDT = int(sum(fds))
    slot_off = np.concatenate([[0], np.cumsum(fds)]).astype(np.int64)

    obs_rank = rank[pid]
    o = np.argsort(obs_rank, kind="stable")          # obs sorted by pose rank
    sorted_rank = obs_rank[o]
    start = np.searchsorted(sorted_rank, np.arange(NPOSE))
    within = np.arange(M, dtype=np.int64) - start[sorted_rank]

    k = sorted_rank >> 7                             # tile index
    part = sorted_rank & 127
    core = k & 7
    slot = k >> 3
    col = slot_off[slot] + within
    # flat position inside [NCORES, P, FDT]
    flat = (core * P + part) * FDT + col

    obs7 = np.zeros((7, NCORES * P * FDT), np.float16)
    obs7[0, flat] = patch_coords[qid[o], 0]
    obs7[1, flat] = patch_coords[qid[o], 1]
    obs7[2, flat] = elevation_angle[qid[o], 0]
    obs7[3, flat] = target_coords[o, 0]
    obs7[4, flat] = target_coords[o, 1]
    obs7[5, flat] = target_coords[o, 2]
    obs7[6, flat] = weights[o, 0]
    obs7 = obs7.reshape(7, NCORES, P, FDT).transpose(1, 0, 2, 3)
    obs7 = np.ascontiguousarray(obs7)

    # pose table per (core, slot, partition)
    pose_of = order.reshape(32, P)                   # [tile, partition]
    tab = np.zeros((NCORES, 7, P, NSLOT), np.float32)
    for s in range(NSLOT):
        for c in range(NCORES):
            pidx = pose_of[8 * s + c]                # [128]
            tab[c, 0:4, :, s] = poses[pidx, 3:7].T   # qx,qy,qz,qw
            tab[c, 4:7, :, s] = poses[pidx, 0:3].T   # tx,ty,tz
    return fds, tab, obs7, o, flat


def kernel(poses, patch_coords, elevation_angle, poses_idx, patch_idx,
           target_coords, weights):
    from concourse.bass_utils import run_bass_kernel_spmd

    fds, tab, obs7, o, flat = _stage(poses, patch_coords, elevation_angle,
                                     poses_idx, patch_idx, target_coords,
                                     weights)
    key = ("main", fds)
    if key not in _CACHE:
        _CACHE[key] = _build_main(fds)
    nc = _CACHE[key]
    res = run_bass_kernel_spmd(
        nc, [{"tab": tab[c], "obs": obs7[c]} for c in range(NCORES)],
        list(range(NCORES)))
    FDT = int(sum(fds))
    outp = np.stack([np.asarray(res.results[c]["out"])
                     for c in range(NCORES)])       # [8, 3, 128, FDT]
    outp = outp.transpose(0, 2, 3, 1).reshape(NCORES * P * FDT, 3)
    res_sorted = outp[flat]                          # [M, 3] in sorted order
    full = np.empty((M, 3), np.float32)
    full[o] = res_sorted.astype(np.float32)
    return full


# revision 4
# speedup vs baseline: 2.0231x; 1.0867x over previous
"""Trainium2 Bass kernel for nn_BoundleAdjustment (2M observations).

Single launch on all 8 NeuronCores.  Host work is pure indexing: observations
are sorted by pose and laid out pose-major, so each SBUF partition holds one
pose's observations per tile-slot and the per-pose rotation/translation become
[P,1] per-partition scalar operands — the 12 R/t streams of the old design
vanish from DMA and from the vector engine.

Device program (per core):
  1. tiny pose-table prologue: q -> R (f32, [128, 4] planes, ~27 ops)
  2. rot per slot: r = R*p + t via tensor_scalar + 2x scalar_tensor_tensor
     with [P,1] f32 scalars, fp16 streams
  3. planar polar+residual in fp16: range/az/el via Abs_reciprocal_sqrt and
     Arctan on the Scalar engine (2 activation-table sets only), quadrant
     fixup with tensor_scalar masks, weighted residual; a few off-critical
     tensor_tensor ops run on GPSIMD to unload the DVE.

Streams are fp16 (2x DVE mode, half the HBM bytes); per-pose scalars stay
f32, which keeps atan2 branch flips rare (rel RMS ~7e-3 vs fp32 reference).
"""

import numpy as np

M = 2097152
NCORES = 8
P = 128
NPOSE = 4096
NSLOT = 4          # tile slots per core; 8 cores * 4 slots * 128 poses = 4096
PI = float(np.pi)

_CACHE = {}


def _build_main(fds):
    import concourse.bass as bass
    import concourse.tile as tile
    from concourse import bacc, mybir

    nc = bacc.Bacc("TRN2", target_bir_lowering=False, debug=False,
                   num_devices=NCORES)
    f32 = mybir.dt.float32
    f16 = mybir.dt.float16
    OP = mybir.AluOpType
    AF = mybir.ActivationFunctionType
    FDT = int(sum(fds))

    tab_d = nc.declare_dram_parameter("tab", [7, P, NSLOT], f32, isOutput=False)
    obs_d = nc.declare_dram_parameter("obs", [7, P, FDT], f16, isOutput=False)
    out_d = nc.declare_dram_parameter("out", [3, P, FDT], f16, isOutput=True)

    with tile.TileContext(nc) as tc:
        with tc.tile_pool(name="pp", bufs=1) as pp:
            vec, act, gp = nc.vector, nc.scalar, nc.gpsimd

            # ---- pose table prologue (f32, FD=4) ----
            tabs = []
            for k in range(7):
                t = pp.tile([P, NSLOT], f32, name=f"tab{k}")
                nc.sync.dma_start(t[:], tab_d[k])
                tabs.append(t)
            qx, qy, qz, qw, tx, ty, tz = tabs

            tcnt = [0]

            def TT4():
                tcnt[0] += 1
                return pp.tile([P, NSLOT], f32, name=f"tt{tcnt[0]}")

            def mul4(a, b):
                d = TT4()
                vec.tensor_tensor(out=d[:], in0=a[:], in1=b[:], op=OP.mult)
                return d

            def add4(a, b):
                d = TT4()
                vec.tensor_tensor(out=d[:], in0=a[:], in1=b[:], op=OP.add)
                return d

            xx = mul4(qx, qx); yy = mul4(qy, qy)
            zz = mul4(qz, qz); ww = mul4(qw, qw)
            xy = mul4(qx, qy); xz = mul4(qx, qz); yz = mul4(qy, qz)
            wx = mul4(qw, qx); wy = mul4(qw, qy); wz = mul4(qw, qz)
            s = add4(add4(xx, yy), add4(zz, ww))
            half = TT4()
            vec.tensor_scalar(out=half[:], in0=s[:], scalar1=0.5, scalar2=None,
                              op0=OP.mult)
            u = TT4()
            vec.reciprocal(u[:], half[:])        # 2/|q|^2

            R = {}

            def diag(m1, m2, nm):
                a = add4(m1, m2)
                b = mul4(a, u)
                d = TT4()
                vec.tensor_scalar(out=d[:], in0=b[:], scalar1=-1.0,
                                  scalar2=1.0, op0=OP.mult, op1=OP.add)
                R[nm] = d

            def offd(m1, m2, op, nm):
                a = TT4()
                vec.tensor_tensor(out=a[:], in0=m1[:], in1=m2[:], op=op)
                R[nm] = mul4(a, u)

            diag(yy, zz, "R00"); diag(xx, zz, "R11"); diag(xx, yy, "R22")
            offd(xy, wz, OP.subtract, "R01"); offd(xz, wy, OP.add, "R02")
            offd(xy, wz, OP.add, "R10"); offd(yz, wx, OP.subtract, "R12")
            offd(xz, wy, OP.subtract, "R20"); offd(yz, wx, OP.add, "R21")

            # ---- obs streams in: p-planes slot-major first, tgt/w later ----
            obs = [pp.tile([P, FDT], f16, name=f"obs{k}") for k in range(7)]
            px, py, pz, X, Y, Z, W = obs
            off = 0
            for s_, fd in enumerate(fds):
                for k in (0, 1, 2):
                    nc.sync.dma_start(obs[k][:, off:off + fd],
                                      obs_d[k, :, off:off + fd])
                off += fd
            for k in (3, 4, 5, 6):
                off = 0
                for s_, fd in enumerate(fds):
                    nc.sync.dma_start(obs[k][:, off:off + fd],
                                      obs_d[k, :, off:off + fd])
                    off += fd

            # ---- rot: r = R*p + t per slot ----
            def rowplane():
                return pp.tile([P, FDT], f16, name=f"rp{tcnt[0]}")

            def rot_row(Rn0, Rn1, Rn2, tcm, nm):
                r = pp.tile([P, FDT], f16, name=nm)
                off = 0
                for s_, fd in enumerate(fds):
                    sl = slice(off, off + fd)
                    a = pp.tile([P, fd], f16, name=f"a{nm}{s_}")
                    vec.tensor_scalar(out=a[:], in0=pz[:, sl],
                                      scalar1=R[Rn2][:, s_:s_ + 1],
                                      scalar2=tcm[:, s_:s_ + 1],
                                      op0=OP.mult, op1=OP.add)
                    b = pp.tile([P, fd], f16, name=f"b{nm}{s_}")
                    vec.scalar_tensor_tensor(out=b[:], in0=py[:, sl],
                                             scalar=R[Rn1][:, s_:s_ + 1],
                                             in1=a[:], op0=OP.mult, op1=OP.add)
                    vec.scalar_tensor_tensor(out=r[:, sl], in0=px[:, sl],
                                             scalar=R[Rn0][:, s_:s_ + 1],
                                             in1=b[:], op0=OP.mult, op1=OP.add)
                    off += fd
                return r

            rx = rot_row("R00", "R01", "R02", tx, "rx")
            ry = rot_row("R10", "R11", "R12", ty, "ry")
            rz = rot_row("R20", "R21", "R22", tz, "rz")

            # ---- polar + residual in 2 column-chunks (slots 01 | 23) ----
            # az = arctan(ry/|rx|)*sign(rx) + pi*[rx<0]*sign(ry)
            chunks = [(0, fds[0] + fds[1]), (fds[0] + fds[1], fds[2] + fds[3])]
            for ci, (c0, cw) in enumerate(chunks):
                cs = slice(c0, c0 + cw)
                last = ci == len(chunks) - 1

                def T():
                    tcnt[0] += 1
                    return pp.tile([P, cw], f16, name=f"pl{tcnt[0]}")

                def tt(a, b, op, eng=vec):
                    d = T()
                    eng.tensor_tensor(out=d[:], in0=a, in1=b, op=op)
                    return d

                def afn(a, fn):
                    d = T()
                    act.activation(d[:], a, fn)
                    return d

                rxc, ryc, rzc = rx[:, cs], ry[:, cs], rz[:, cs]
                sqx = tt(rxc, rxc, OP.mult)
                sqy = tt(ryc, ryc, OP.mult)
                sqz = tt(rzc, rzc, OP.mult, eng=gp)
                rho2 = tt(sqx[:], sqy[:], OP.add)
                r2 = tt(rho2[:], sqz[:], OP.add)
                # ACT set 1: abs_reciprocal_sqrt
                u2 = afn(rho2[:], AF.Abs_reciprocal_sqrt)    # 1/rho
                ur = afn(r2[:], AF.Abs_reciprocal_sqrt)      # 1/r
                ux = afn(rxc, AF.Abs_reciprocal_sqrt)        # 1/sqrt(|rx|)
                ux2 = tt(ux[:], ux[:], OP.mult)              # 1/|rx|
                rng = tt(r2[:], ur[:], OP.mult)              # r
                e1 = tt(rzc, u2[:], OP.mult)
                t0 = tt(ryc, ux2[:], OP.mult)                # ry/|rx|
                # ACT set 2: arctan + sign
                el = afn(e1[:], AF.Arctan)
                azp = afn(t0[:], AF.Arctan)
                sgy = afn(ryc, AF.Sign)
                sgx = afn(rxc, AF.Sign)
                m = T()
                vec.tensor_scalar(out=m[:], in0=rxc, scalar1=0.0, scalar2=PI,
                                  op0=OP.is_lt, op1=OP.mult)
                az1 = tt(azp[:], sgx[:], OP.mult)
                corr = tt(m[:], sgy[:], OP.mult, eng=gp)
                az = tt(az1[:], corr[:], OP.add)

                for i, (pcomp, tgt) in enumerate(
                        ((rng, X[:, cs]), (az, Y[:, cs]), (el, Z[:, cs]))):
                    d = tt(pcomp[:], tgt, OP.subtract)
                    o = tt(d[:], W[:, cs], OP.mult,
                           eng=gp if (not last and i != 1) else vec)
                    nc.sync.dma_start(out_d[i, :, cs], o[:])
    nc.finalize()
    return nc


def _stage(poses, patch_coords, elevation_angle, poses_idx, patch_idx,
           target_coords, weights):
    """Pure-indexing host staging.  Returns (fds, tab array, obs array,
    gather indices for unpadding)."""
    poses = np.asarray(poses, dtype=np.float32)
    patch_coords = np.asarray(patch_coords, dtype=np.float32)
    elevation_angle = np.asarray(elevation_angle, dtype=np.float32)
    target_coords = np.asarray(target_coords, dtype=np.float32)
    weights = np.asarray(weights, dtype=np.float32)
    pid = np.asarray(poses_idx).astype(np.int64)
    qid = np.asarray(patch_idx).astype(np.int64)

    cnt = np.bincount(pid, minlength=NPOSE)
    order = np.argsort(-cnt, kind="stable")          # poses by count desc
    rank = np.empty(NPOSE, np.int64)
    rank[order] = np.arange(NPOSE)
    # tile k = ranks [128k,128(k+1)); tile -> core k%8, slot k//8
    # slot FD = count of first pose of tile 8s (max across cores)
    fds = tuple(int(cnt[order[128 * (8 * s)]]) for s in range(NSLOT))
    FDT = int(sum(fds))
    slot_off = np.concatenate([[0], np.cumsum(fds)]).astype(np.int64)

    obs_rank = rank[pid]
    o = np.argsort(obs_rank, kind="stable")          # obs sorted by pose rank
    sorted_rank = obs_rank[o]
    start = np.searchsorted(sorted_rank, np.arange(NPOSE))
    within = np.arange(M, dtype=np.int64) - start[sorted_rank]

    k = sorted_rank >> 7                             # tile index
    part = sorted_rank & 127
    core = k & 7
    slot = k >> 3
    col = slot_off[slot] + within
    # flat position inside [NCORES, P, FDT]
    flat = (core * P + part) * FDT + col

    obs7 = np.zeros((7, NCORES * P * FDT), np.float16)
    obs7[0, flat] = patch_coords[qid[o], 0]
    obs7[1, flat] = patch_coords[qid[o], 1]
    obs7[2, flat] = elevation_angle[qid[o], 0]
    obs7[3, flat] = target_coords[o, 0]
    obs7[4, flat] = target_coords[o, 1]
    obs7[5, flat] = target_coords[o, 2]
    obs7[6, flat] = weights[o, 0]
    obs7 = obs7.reshape(7, NCORES, P, FDT).transpose(1, 0, 2, 3)
    obs7 = np.ascontiguousarray(obs7)

    # pose table per (core, slot, partition)
    pose_of = order.reshape(32, P)                   # [tile, partition]
    tab = np.zeros((NCORES, 7, P, NSLOT), np.float32)
    for s in range(NSLOT):
        for c in range(NCORES):
            pidx = pose_of[8 * s + c]                # [128]
            tab[c, 0:4, :, s] = poses[pidx, 3:7].T   # qx,qy,qz,qw
            tab[c, 4:7, :, s] = poses[pidx, 0:3].T   # tx,ty,tz
    return fds, tab, obs7, o, flat


def kernel(poses, patch_coords, elevation_angle, poses_idx, patch_idx,
           target_coords, weights):
    from concourse.bass_utils import run_bass_kernel_spmd

    fds, tab, obs7, o, flat = _stage(poses, patch_coords, elevation_angle,
                                     poses_idx, patch_idx, target_coords,
                                     weights)
    key = ("main", fds)
    if key not in _CACHE:
        _CACHE[key] = _build_main(fds)
    nc = _CACHE[key]
    res = run_bass_kernel_spmd(
        nc, [{"tab": tab[c], "obs": obs7[c]} for c in range(NCORES)],
        list(range(NCORES)))
    FDT = int(sum(fds))
    outp = np.stack([np.asarray(res.results[c]["out"])
                     for c in range(NCORES)])       # [8, 3, 128, FDT]
    outp = outp.transpose(0, 2, 3, 1).reshape(NCORES * P * FDT, 3)
    res_sorted = outp[flat]                          # [M, 3] in sorted order
    full = np.empty((M, 3), np.float32)
    full[o] = res_sorted.astype(np.float32)
    return full


# revision 5
# speedup vs baseline: 2.1236x; 1.0497x over previous
"""Trainium2 Bass kernel for nn_BoundleAdjustment (2M observations).

Single launch on all 8 NeuronCores.  Host work is pure indexing: observations
are sorted by pose and laid out pose-major, so each SBUF partition holds one
pose's observations per tile-slot and the per-pose rotation/translation become
[P,1] per-partition scalar operands — the 12 R/t streams of the old design
vanish from DMA and from the vector engine.

Device program (per core):
  1. tiny pose-table prologue: q -> R (f32, [128, 4] planes, ~27 ops)
  2. rot per slot: r = R*p + t via tensor_scalar + 2x scalar_tensor_tensor
     with [P,1] f32 scalars, fp16 streams
  3. planar polar+residual in fp16: range/az/el via Abs_reciprocal_sqrt and
     Arctan on the Scalar engine (2 activation-table sets only), quadrant
     fixup with tensor_scalar masks, weighted residual; a few off-critical
     tensor_tensor ops run on GPSIMD to unload the DVE.

Streams are fp16 (2x DVE mode, half the HBM bytes); per-pose scalars stay
f32, which keeps atan2 branch flips rare (rel RMS ~7e-3 vs fp32 reference).
"""

import numpy as np

M = 2097152
NCORES = 8
P = 128
NPOSE = 4096
NSLOT = 4          # tile slots per core; 8 cores * 4 slots * 128 poses = 4096
PI = float(np.pi)

_CACHE = {}


def _build_main(fds):
    import concourse.bass as bass
    import concourse.tile as tile
    from concourse import bacc, mybir

    nc = bacc.Bacc("TRN2", target_bir_lowering=False, debug=False,
                   num_devices=NCORES)
    f32 = mybir.dt.float32
    f16 = mybir.dt.float16
    OP = mybir.AluOpType
    AF = mybir.ActivationFunctionType
    FDT = int(sum(fds))

    tab_d = nc.declare_dram_parameter("tab", [7, P, NSLOT], f32, isOutput=False)
    obs_d = nc.declare_dram_parameter("obs", [7, P, FDT], f16, isOutput=False)
    out_d = nc.declare_dram_parameter("out", [3, P, FDT], f16, isOutput=True)

    with tile.TileContext(nc) as tc:
        with tc.tile_pool(name="pp", bufs=1) as pp:
            vec, act, gp = nc.vector, nc.scalar, nc.gpsimd

            # ---- pose table prologue (f32, FD=4) ----
            tabs = []
            for k in range(7):
                t = pp.tile([P, NSLOT], f32, name=f"tab{k}")
                nc.sync.dma_start(t[:], tab_d[k])
                tabs.append(t)
            qx, qy, qz, qw, tx, ty, tz = tabs

            tcnt = [0]

            def TT4():
                tcnt[0] += 1
                return pp.tile([P, NSLOT], f32, name=f"tt{tcnt[0]}")

            def mul4(a, b):
                d = TT4()
                vec.tensor_tensor(out=d[:], in0=a[:], in1=b[:], op=OP.mult)
                return d

            def add4(a, b):
                d = TT4()
                vec.tensor_tensor(out=d[:], in0=a[:], in1=b[:], op=OP.add)
                return d

            xx = mul4(qx, qx); yy = mul4(qy, qy)
            zz = mul4(qz, qz); ww = mul4(qw, qw)
            xy = mul4(qx, qy); xz = mul4(qx, qz); yz = mul4(qy, qz)
            wx = mul4(qw, qx); wy = mul4(qw, qy); wz = mul4(qw, qz)
            s = add4(add4(xx, yy), add4(zz, ww))
            half = TT4()
            vec.tensor_scalar(out=half[:], in0=s[:], scalar1=0.5, scalar2=None,
                              op0=OP.mult)
            u = TT4()
            vec.reciprocal(u[:], half[:])        # 2/|q|^2

            R = {}

            def diag(m1, m2, nm):
                a = add4(m1, m2)
                b = mul4(a, u)
                d = TT4()
                vec.tensor_scalar(out=d[:], in0=b[:], scalar1=-1.0,
                                  scalar2=1.0, op0=OP.mult, op1=OP.add)
                R[nm] = d

            def offd(m1, m2, op, nm):
                a = TT4()
                vec.tensor_tensor(out=a[:], in0=m1[:], in1=m2[:], op=op)
                R[nm] = mul4(a, u)

            diag(yy, zz, "R00"); diag(xx, zz, "R11"); diag(xx, yy, "R22")
            offd(xy, wz, OP.subtract, "R01"); offd(xz, wy, OP.add, "R02")
            offd(xy, wz, OP.add, "R10"); offd(yz, wx, OP.subtract, "R12")
            offd(xz, wy, OP.subtract, "R20"); offd(yz, wx, OP.add, "R21")

            # ---- obs streams in: p-planes slot-major first, tgt/w later ----
            obs = [pp.tile([P, FDT], f16, name=f"obs{k}") for k in range(7)]
            px, py, pz, X, Y, Z, W = obs
            off = 0
            for s_, fd in enumerate(fds):
                for k in (0, 1, 2):
                    nc.sync.dma_start(obs[k][:, off:off + fd],
                                      obs_d[k, :, off:off + fd])
                off += fd
            for k in (3, 4, 5, 6):
                off = 0
                for s_, fd in enumerate(fds):
                    nc.sync.dma_start(obs[k][:, off:off + fd],
                                      obs_d[k, :, off:off + fd])
                    off += fd

            # ---- rot: r = R*p + t per slot ----
            def rowplane():
                return pp.tile([P, FDT], f16, name=f"rp{tcnt[0]}")

            def rot_row(Rn0, Rn1, Rn2, tcm, nm):
                r = pp.tile([P, FDT], f16, name=nm)
                off = 0
                for s_, fd in enumerate(fds):
                    sl = slice(off, off + fd)
                    a = pp.tile([P, fd], f16, name=f"a{nm}{s_}")
                    vec.tensor_scalar(out=a[:], in0=pz[:, sl],
                                      scalar1=R[Rn2][:, s_:s_ + 1],
                                      scalar2=tcm[:, s_:s_ + 1],
                                      op0=OP.mult, op1=OP.add)
                    b = pp.tile([P, fd], f16, name=f"b{nm}{s_}")
                    vec.scalar_tensor_tensor(out=b[:], in0=py[:, sl],
                                             scalar=R[Rn1][:, s_:s_ + 1],
                                             in1=a[:], op0=OP.mult, op1=OP.add)
                    vec.scalar_tensor_tensor(out=r[:, sl], in0=px[:, sl],
                                             scalar=R[Rn0][:, s_:s_ + 1],
                                             in1=b[:], op0=OP.mult, op1=OP.add)
                    off += fd
                return r

            rx = rot_row("R00", "R01", "R02", tx, "rx")
            ry = rot_row("R10", "R11", "R12", ty, "ry")
            rz = rot_row("R20", "R21", "R22", tz, "rz")

            # ---- polar + residual in 2 column-chunks (slots 01 | 23) ----
            # az = arctan(ry/|rx|)*sign(rx) + pi*[rx<0]*sign(ry)
            chunks = [(0, fds[0] + fds[1]), (fds[0] + fds[1], fds[2] + fds[3])]
            for ci, (c0, cw) in enumerate(chunks):
                cs = slice(c0, c0 + cw)
                last = ci == len(chunks) - 1

                def T():
                    tcnt[0] += 1
                    return pp.tile([P, cw], f16, name=f"pl{tcnt[0]}")

                def tt(a, b, op, eng=vec):
                    d = T()
                    eng.tensor_tensor(out=d[:], in0=a, in1=b, op=op)
                    return d

                def afn(a, fn):
                    d = T()
                    act.activation(d[:], a, fn)
                    return d

                rxc, ryc, rzc = rx[:, cs], ry[:, cs], rz[:, cs]
                sqx = tt(rxc, rxc, OP.mult)
                sqy = tt(ryc, ryc, OP.mult)
                sqz = tt(rzc, rzc, OP.mult, eng=gp)
                rho2 = tt(sqx[:], sqy[:], OP.add)
                r2 = tt(rho2[:], sqz[:], OP.add)
                # ACT set 1: abs_reciprocal_sqrt
                u2 = afn(rho2[:], AF.Abs_reciprocal_sqrt)    # 1/rho
                ur = afn(r2[:], AF.Abs_reciprocal_sqrt)      # 1/r
                ux = afn(rxc, AF.Abs_reciprocal_sqrt)        # 1/sqrt(|rx|)
                ux2 = tt(ux[:], ux[:], OP.mult)              # 1/|rx|
                rng = tt(r2[:], ur[:], OP.mult)              # r
                e1 = tt(rzc, u2[:], OP.mult)
                t0 = tt(ryc, ux2[:], OP.mult)                # ry/|rx|
                # ACT set 2: arctan + sign
                el = afn(e1[:], AF.Arctan)
                azp = afn(t0[:], AF.Arctan)
                sgy = afn(ryc, AF.Sign)
                sgx = afn(rxc, AF.Sign)
                m = T()
                vec.tensor_scalar(out=m[:], in0=rxc, scalar1=0.0, scalar2=PI,
                                  op0=OP.is_lt, op1=OP.mult)
                az1 = tt(azp[:], sgx[:], OP.mult)
                corr = tt(m[:], sgy[:], OP.mult, eng=gp)
                az = tt(az1[:], corr[:], OP.add)

                for i, (pcomp, tgt) in enumerate(
                        ((rng, X[:, cs]), (az, Y[:, cs]), (el, Z[:, cs]))):
                    d = tt(pcomp[:], tgt, OP.subtract)
                    o = tt(d[:], W[:, cs], OP.mult,
                           eng=gp if (not last and i != 1) else vec)
                    nc.sync.dma_start(out_d[i, :, cs], o[:])
    nc.finalize()
    return nc


def _stage(poses, patch_coords, elevation_angle, poses_idx, patch_idx,
           target_coords, weights):
    """Pure-indexing host staging.  Returns (fds, tab array, obs array,
    gather indices for unpadding)."""
    poses = np.asarray(poses, dtype=np.float32)
    patch_coords = np.asarray(patch_coords, dtype=np.float32)
    elevation_angle = np.asarray(elevation_angle, dtype=np.float32)
    target_coords = np.asarray(target_coords, dtype=np.float32)
    weights = np.asarray(weights, dtype=np.float32)
    pid = np.asarray(poses_idx).astype(np.int64)
    qid = np.asarray(patch_idx).astype(np.int64)

    cnt = np.bincount(pid, minlength=NPOSE)
    order = np.argsort(-cnt, kind="stable")          # poses by count desc
    rank = np.empty(NPOSE, np.int64)
    rank[order] = np.arange(NPOSE)
    # tile k = ranks [128k,128(k+1)); tile -> core k%8, slot k//8
    # slot FD = count of first pose of tile 8s (max across cores)
    fds = tuple((int(cnt[order[128 * (8 * s)]]) + 7) & ~7 for s in range(NSLOT))
    FDT = int(sum(fds))
    slot_off = np.concatenate([[0], np.cumsum(fds)]).astype(np.int64)

    obs_rank = rank[pid]
    o = np.argsort(obs_rank, kind="stable")          # obs sorted by pose rank
    sorted_rank = obs_rank[o]
    start = np.searchsorted(sorted_rank, np.arange(NPOSE))
    within = np.arange(M, dtype=np.int64) - start[sorted_rank]

    k = sorted_rank >> 7                             # tile index
    part = sorted_rank & 127
    core = k & 7
    slot = k >> 3
    col = slot_off[slot] + within
    # flat position inside [NCORES, P, FDT]
    flat = (core * P + part) * FDT + col

    obs7 = np.zeros((7, NCORES * P * FDT), np.float16)
    obs7[0, flat] = patch_coords[qid[o], 0]
    obs7[1, flat] = patch_coords[qid[o], 1]
    obs7[2, flat] = elevation_angle[qid[o], 0]
    obs7[3, flat] = target_coords[o, 0]
    obs7[4, flat] = target_coords[o, 1]
    obs7[5, flat] = target_coords[o, 2]
    obs7[6, flat] = weights[o, 0]
    obs7 = obs7.reshape(7, NCORES, P, FDT).transpose(1, 0, 2, 3)
    obs7 = np.ascontiguousarray(obs7)

    # pose table per (core, slot, partition)
    pose_of = order.reshape(32, P)                   # [tile, partition]
    tab = np.zeros((NCORES, 7, P, NSLOT), np.float32)
    for s in range(NSLOT):
        for c in range(NCORES):
            pidx = pose_of[8 * s + c]                # [128]
            tab[c, 0:4, :, s] = poses[pidx, 3:7].T   # qx,qy,qz,qw
            tab[c, 4:7, :, s] = poses[pidx, 0:3].T   # tx,ty,tz
    return fds, tab, obs7, o, flat


def kernel(poses, patch_coords, elevation_angle, poses_idx, patch_idx,
           target_coords, weights):
    from concourse.bass_utils import run_bass_kernel_spmd

    fds, tab, obs7, o, flat = _stage(poses, patch_coords, elevation_angle,
                                     poses_idx, patch_idx, target_coords,
                                     weights)
    key = ("main", fds)
    if key not in _CACHE:
        _CACHE[key] = _build_main(fds)
    nc = _CACHE[key]
    res = run_bass_kernel_spmd(
        nc, [{"tab": tab[c], "obs": obs7[c]} for c in range(NCORES)],
        list(range(NCORES)))
    FDT = int(sum(fds))
    outp = np.stack([np.asarray(res.results[c]["out"])
                     for c in range(NCORES)])       # [8, 3, 128, FDT]
    outp = outp.transpose(0, 2, 3, 1).reshape(NCORES * P * FDT, 3)
    res_sorted = outp[flat]                          # [M, 3] in sorted order
    full = np.empty((M, 3), np.float32)
    full[o] = res_sorted.astype(np.float32)
    return full


# revision 6
# speedup vs baseline: 2.3061x; 1.0860x over previous
"""Trainium2 Bass kernel for nn_BoundleAdjustment (2M observations).

Single launch on all 8 NeuronCores.  Host work is pure indexing: observations
are sorted by pose and laid out pose-major, so each SBUF partition holds one
pose's observations per tile-slot and the per-pose rotation/translation become
[P,1] per-partition scalar operands — the 12 R/t streams of the old design
vanish from DMA and from the vector engine.

Device program (per core):
  1. tiny pose-table prologue: q -> R (f32, [128, 4] planes, ~27 ops)
  2. rot per slot: r = R*p + t via tensor_scalar + 2x scalar_tensor_tensor
     with [P,1] f32 scalars, fp16 streams
  3. planar polar+residual in fp16: range/az/el via Abs_reciprocal_sqrt and
     Arctan on the Scalar engine (2 activation-table sets only), quadrant
     fixup with tensor_scalar masks, weighted residual; a few off-critical
     tensor_tensor ops run on GPSIMD to unload the DVE.

Streams are fp16 (2x DVE mode, half the HBM bytes); per-pose scalars stay
f32, which keeps atan2 branch flips rare (rel RMS ~7e-3 vs fp32 reference).
"""

import numpy as np

M = 2097152
NCORES = 8
P = 128
NPOSE = 4096
NSLOT = 4          # tile slots per core; 8 cores * 4 slots * 128 poses = 4096
PI = float(np.pi)

_CACHE = {}


def _build_main(fds):
    import concourse.bass as bass
    import concourse.tile as tile
    from concourse import bacc, mybir

    nc = bacc.Bacc("TRN2", target_bir_lowering=False, debug=False,
                   num_devices=NCORES)
    f32 = mybir.dt.float32
    f16 = mybir.dt.float16
    OP = mybir.AluOpType
    AF = mybir.ActivationFunctionType
    FDT = int(sum(fds))

    tab_d = nc.declare_dram_parameter("tab", [7, P, NSLOT], f32, isOutput=False)
    obs_d = nc.declare_dram_parameter("obs", [7, P, FDT], f16, isOutput=False)
    out_d = nc.declare_dram_parameter("out", [3, P, FDT], f16, isOutput=True)

    with tile.TileContext(nc) as tc:
        with tc.tile_pool(name="pp", bufs=1) as pp:
            vec, act, gp = nc.vector, nc.scalar, nc.gpsimd

            # ---- pose table prologue (f32, FD=4) ----
            tabs = []
            for k in range(7):
                t = pp.tile([P, NSLOT], f32, name=f"tab{k}")
                nc.sync.dma_start(t[:], tab_d[k])
                tabs.append(t)
            qx, qy, qz, qw, tx, ty, tz = tabs

            tcnt = [0]

            def TT4():
                tcnt[0] += 1
                return pp.tile([P, NSLOT], f32, name=f"tt{tcnt[0]}")

            def mul4(a, b):
                d = TT4()
                vec.tensor_tensor(out=d[:], in0=a[:], in1=b[:], op=OP.mult)
                return d

            def add4(a, b):
                d = TT4()
                vec.tensor_tensor(out=d[:], in0=a[:], in1=b[:], op=OP.add)
                return d

            xx = mul4(qx, qx); yy = mul4(qy, qy)
            zz = mul4(qz, qz); ww = mul4(qw, qw)
            xy = mul4(qx, qy); xz = mul4(qx, qz); yz = mul4(qy, qz)
            wx = mul4(qw, qx); wy = mul4(qw, qy); wz = mul4(qw, qz)
            s = add4(add4(xx, yy), add4(zz, ww))
            half = TT4()
            vec.tensor_scalar(out=half[:], in0=s[:], scalar1=0.5, scalar2=None,
                              op0=OP.mult)
            u = TT4()
            vec.reciprocal(u[:], half[:])        # 2/|q|^2

            R = {}

            def diag(m1, m2, nm):
                a = add4(m1, m2)
                b = mul4(a, u)
                d = TT4()
                vec.tensor_scalar(out=d[:], in0=b[:], scalar1=-1.0,
                                  scalar2=1.0, op0=OP.mult, op1=OP.add)
                R[nm] = d

            def offd(m1, m2, op, nm):
                a = TT4()
                vec.tensor_tensor(out=a[:], in0=m1[:], in1=m2[:], op=op)
                R[nm] = mul4(a, u)

            diag(yy, zz, "R00"); diag(xx, zz, "R11"); diag(xx, yy, "R22")
            offd(xy, wz, OP.subtract, "R01"); offd(xz, wy, OP.add, "R02")
            offd(xy, wz, OP.add, "R10"); offd(yz, wx, OP.subtract, "R12")
            offd(xz, wy, OP.subtract, "R20"); offd(yz, wx, OP.add, "R21")

            # ---- obs streams in: p-planes slot-major first, tgt/w later ----
            obs = [pp.tile([P, FDT], f16, name=f"obs{k}") for k in range(7)]
            px, py, pz, X, Y, Z, W = obs
            off = 0
            for s_, fd in enumerate(fds):
                for k in (0, 1, 2):
                    nc.sync.dma_start(obs[k][:, off:off + fd],
                                      obs_d[k, :, off:off + fd])
                off += fd
            for k in (3, 4, 5, 6):
                off = 0
                for s_, fd in enumerate(fds):
                    nc.sync.dma_start(obs[k][:, off:off + fd],
                                      obs_d[k, :, off:off + fd])
                    off += fd

            # ---- rot: r = R*p + t per slot ----
            def rowplane():
                return pp.tile([P, FDT], f16, name=f"rp{tcnt[0]}")

            def rot_row(Rn0, Rn1, Rn2, tcm, nm):
                r = pp.tile([P, FDT], f16, name=nm)
                off = 0
                for s_, fd in enumerate(fds):
                    sl = slice(off, off + fd)
                    a = pp.tile([P, fd], f16, name=f"a{nm}{s_}")
                    vec.tensor_scalar(out=a[:], in0=pz[:, sl],
                                      scalar1=R[Rn2][:, s_:s_ + 1],
                                      scalar2=tcm[:, s_:s_ + 1],
                                      op0=OP.mult, op1=OP.add)
                    b = pp.tile([P, fd], f16, name=f"b{nm}{s_}")
                    vec.scalar_tensor_tensor(out=b[:], in0=py[:, sl],
                                             scalar=R[Rn1][:, s_:s_ + 1],
                                             in1=a[:], op0=OP.mult, op1=OP.add)
                    vec.scalar_tensor_tensor(out=r[:, sl], in0=px[:, sl],
                                             scalar=R[Rn0][:, s_:s_ + 1],
                                             in1=b[:], op0=OP.mult, op1=OP.add)
                    off += fd
                return r

            rx = rot_row("R00", "R01", "R02", tx, "rx")
            ry = rot_row("R10", "R11", "R12", ty, "ry")
            rz = rot_row("R20", "R21", "R22", tz, "rz")

            # ---- polar + residual in 2 column-chunks (slots 01 | 23) ----
            # az = arctan(ry/|rx|)*sign(rx) + pi*[rx<0]*sign(ry)
            # ACT emission order is phase-merged (ARS A, ARS B, trig A+B)
            # so the activation-table set loads only 3 times total.
            chunks = [(0, fds[0] + fds[1]), (fds[0] + fds[1], fds[2] + fds[3])]

            def T(cw):
                tcnt[0] += 1
                return pp.tile([P, cw], f16, name=f"pl{tcnt[0]}")

            def tt(cw, a, b, op, eng=vec):
                d = T(cw)
                eng.tensor_tensor(out=d[:], in0=a, in1=b, op=op)
                return d

            def afn(cw, a, fn):
                d = T(cw)
                act.activation(d[:], a, fn)
                return d

            st = []
            for ci, (c0, cw) in enumerate(chunks):
                cs = slice(c0, c0 + cw)
                rxc, ryc, rzc = rx[:, cs], ry[:, cs], rz[:, cs]
                sqx = tt(cw, rxc, rxc, OP.mult)
                sqy = tt(cw, ryc, ryc, OP.mult)
                sqz = tt(cw, rzc, rzc, OP.mult)
                rho2 = tt(cw, sqx[:], sqy[:], OP.add)
                r2 = tt(cw, rho2[:], sqz[:], OP.add)
                # ACT set 1: abs_reciprocal_sqrt
                u2 = afn(cw, rho2[:], AF.Abs_reciprocal_sqrt)   # 1/rho
                ur = afn(cw, r2[:], AF.Abs_reciprocal_sqrt)     # 1/r
                ux = afn(cw, rxc, AF.Abs_reciprocal_sqrt)       # 1/sqrt|rx|
                st.append((cs, cw, rxc, ryc, rzc, r2, u2, ur, ux))

            for ci, (cs, cw, rxc, ryc, rzc, r2, u2, ur, ux) in enumerate(st):
                last = ci == len(st) - 1
                ux2 = tt(cw, ux[:], ux[:], OP.mult)             # 1/|rx|
                rng = tt(cw, r2[:], ur[:], OP.mult)             # r
                e1 = tt(cw, rzc, u2[:], OP.mult)
                t0 = tt(cw, ryc, ux2[:], OP.mult)               # ry/|rx|
                # ACT set 2: arctan + sign
                el = afn(cw, e1[:], AF.Arctan)
                azp = afn(cw, t0[:], AF.Arctan)
                sgy = afn(cw, ryc, AF.Sign)
                sgx = afn(cw, rxc, AF.Sign)
                m = T(cw)
                vec.tensor_scalar(out=m[:], in0=rxc, scalar1=0.0, scalar2=PI,
                                  op0=OP.is_lt, op1=OP.mult)
                az1 = tt(cw, azp[:], sgx[:], OP.mult)
                corr = tt(cw, m[:], sgy[:], OP.mult, eng=gp)
                az = tt(cw, az1[:], corr[:], OP.add)

                for i, (pcomp, tgt) in enumerate(
                        ((rng, X[:, cs]), (az, Y[:, cs]), (el, Z[:, cs]))):
                    d = tt(cw, pcomp[:], tgt, OP.subtract)
                    o = tt(cw, d[:], W[:, cs], OP.mult,
                           eng=gp if (not last and i != 1) else vec)
                    nc.sync.dma_start(out_d[i, :, cs], o[:])
    nc.finalize()
    return nc


def _stage(poses, patch_coords, elevation_angle, poses_idx, patch_idx,
           target_coords, weights):
    """Pure-indexing host staging.  Returns (fds, tab array, obs array,
    gather indices for unpadding)."""
    poses = np.asarray(poses, dtype=np.float32)
    patch_coords = np.asarray(patch_coords, dtype=np.float32)
    elevation_angle = np.asarray(elevation_angle, dtype=np.float32)
    target_coords = np.asarray(target_coords, dtype=np.float32)
    weights = np.asarray(weights, dtype=np.float32)
    pid = np.asarray(poses_idx).astype(np.int64)
    qid = np.asarray(patch_idx).astype(np.int64)

    cnt = np.bincount(pid, minlength=NPOSE)
    order = np.argsort(-cnt, kind="stable")          # poses by count desc
    rank = np.empty(NPOSE, np.int64)
    rank[order] = np.arange(NPOSE)
    # tile k = ranks [128k,128(k+1)); tile -> core k%8, slot k//8
    # slot FD = count of first pose of tile 8s (max across cores)
    fds = tuple((int(cnt[order[128 * (8 * s)]]) + 7) & ~7 for s in range(NSLOT))
    FDT = int(sum(fds))
    slot_off = np.concatenate([[0], np.cumsum(fds)]).astype(np.int64)

    obs_rank = rank[pid]
    o = np.argsort(obs_rank, kind="stable")          # obs sorted by pose rank
    sorted_rank = obs_rank[o]
    start = np.searchsorted(sorted_rank, np.arange(NPOSE))
    within = np.arange(M, dtype=np.int64) - start[sorted_rank]

    k = sorted_rank >> 7                             # tile index
    part = sorted_rank & 127
    core = k & 7
    slot = k >> 3
    col = slot_off[slot] + within
    # flat position inside [NCORES, P, FDT]
    flat = (core * P + part) * FDT + col

    obs7 = np.zeros((7, NCORES * P * FDT), np.float16)
    obs7[0, flat] = patch_coords[qid[o], 0]
    obs7[1, flat] = patch_coords[qid[o], 1]
    obs7[2, flat] = elevation_angle[qid[o], 0]
    obs7[3, flat] = target_coords[o, 0]
    obs7[4, flat] = target_coords[o, 1]
    obs7[5, flat] = target_coords[o, 2]
    obs7[6, flat] = weights[o, 0]
    obs7 = obs7.reshape(7, NCORES, P, FDT).transpose(1, 0, 2, 3)
    obs7 = np.ascontiguousarray(obs7)

    # pose table per (core, slot, partition)
    pose_of = order.reshape(32, P)                   # [tile, partition]
    tab = np.zeros((NCORES, 7, P, NSLOT), np.float32)
    for s in range(NSLOT):
        for c in range(NCORES):
            pidx = pose_of[8 * s + c]                # [128]
            tab[c, 0:4, :, s] = poses[pidx, 3:7].T   # qx,qy,qz,qw
            tab[c, 4:7, :, s] = poses[pidx, 0:3].T   # tx,ty,tz
    return fds, tab, obs7, o, flat


def kernel(poses, patch_coords, elevation_angle, poses_idx, patch_idx,
           target_coords, weights):
    from concourse.bass_utils import run_bass_kernel_spmd

    fds, tab, obs7, o, flat = _stage(poses, patch_coords, elevation_angle,
                                     poses_idx, patch_idx, target_coords,
                                     weights)
    key = ("main", fds)
    if key not in _CACHE:
        _CACHE[key] = _build_main(fds)
    nc = _CACHE[key]
    res = run_bass_kernel_spmd(
        nc, [{"tab": tab[c], "obs": obs7[c]} for c in range(NCORES)],
        list(range(NCORES)))
    FDT = int(sum(fds))
    outp = np.stack([np.asarray(res.results[c]["out"])
                     for c in range(NCORES)])       # [8, 3, 128, FDT]
    outp = outp.transpose(0, 2, 3, 1).reshape(NCORES * P * FDT, 3)
    res_sorted = outp[flat]                          # [M, 3] in sorted order
    full = np.empty((M, 3), np.float32)
    full[o] = res_sorted.astype(np.float32)
    return full


# revision 7
# speedup vs baseline: 2.4601x; 1.0668x over previous
"""Trainium2 Bass kernel for nn_BoundleAdjustment (2M observations).

Single launch on all 8 NeuronCores.  Host work is pure indexing: observations
are sorted by pose and laid out pose-major, so each SBUF partition holds one
pose's observations per tile-slot and the per-pose rotation/translation become
[P,1] per-partition scalar operands — the 12 R/t streams of the old design
vanish from DMA and from the vector engine.

Device program (per core):
  1. tiny pose-table prologue: q -> R (f32, [128, 4] planes, ~27 ops)
  2. rot per slot: r = R*p + t via tensor_scalar + 2x scalar_tensor_tensor
     with [P,1] f32 scalars, fp16 streams
  3. planar polar+residual in fp16: range/az/el via Abs_reciprocal_sqrt and
     Arctan on the Scalar engine (2 activation-table sets only), quadrant
     fixup with tensor_scalar masks, weighted residual; a few off-critical
     tensor_tensor ops run on GPSIMD to unload the DVE.

Streams are fp16 (2x DVE mode, half the HBM bytes); per-pose scalars stay
f32, which keeps atan2 branch flips rare (rel RMS ~7e-3 vs fp32 reference).
"""

import numpy as np

M = 2097152
NCORES = 8
P = 128
NPOSE = 4096
NSLOT = 4          # tile slots per core; 8 cores * 4 slots * 128 poses = 4096
PI = float(np.pi)

_CACHE = {}


def _build_main(fds):
    import concourse.bass as bass
    import concourse.tile as tile
    from concourse import bacc, mybir

    nc = bacc.Bacc("TRN2", target_bir_lowering=False, debug=False,
                   num_devices=NCORES)
    f32 = mybir.dt.float32
    f16 = mybir.dt.float16
    OP = mybir.AluOpType
    AF = mybir.ActivationFunctionType
    FDT = int(sum(fds))

    tab_d = nc.declare_dram_parameter("tab", [P, 7 * NSLOT], f32,
                                      isOutput=False)
    obs_d = nc.declare_dram_parameter("obs", [7, P, FDT], f16, isOutput=False)
    out_d = nc.declare_dram_parameter("out", [3, P, FDT], f16, isOutput=True)

    with tile.TileContext(nc) as tc:
        with tc.tile_pool(name="pp", bufs=1) as pp:
            vec, act, gp = nc.vector, nc.scalar, nc.gpsimd

            # ---- pose table prologue (f32, FD=4; one [P,28] DMA) ----
            tabt = pp.tile([P, 7 * NSLOT], f32, name="tabt")
            nc.sync.dma_start(tabt[:], tab_d[:, :])
            qx, qy, qz, qw, tx, ty, tz = [
                tabt[:, 4 * k:4 * k + 4] for k in range(7)]

            tcnt = [0]

            def TT4():
                tcnt[0] += 1
                return pp.tile([P, NSLOT], f32, name=f"tt{tcnt[0]}")

            def _ap(x):
                return x if type(x).__name__ == "AP" else x[:]

            def mul4(a, b):
                d = TT4()
                vec.tensor_tensor(out=d[:], in0=_ap(a), in1=_ap(b), op=OP.mult)
                return d

            def add4(a, b):
                d = TT4()
                vec.tensor_tensor(out=d[:], in0=_ap(a), in1=_ap(b), op=OP.add)
                return d

            xx = mul4(qx, qx); yy = mul4(qy, qy)
            zz = mul4(qz, qz); ww = mul4(qw, qw)
            xy = mul4(qx, qy); xz = mul4(qx, qz); yz = mul4(qy, qz)
            wx = mul4(qw, qx); wy = mul4(qw, qy); wz = mul4(qw, qz)
            s = add4(add4(xx, yy), add4(zz, ww))
            half = TT4()
            vec.tensor_scalar(out=half[:], in0=s[:], scalar1=0.5, scalar2=None,
                              op0=OP.mult)
            u = TT4()
            vec.reciprocal(u[:], half[:])        # 2/|q|^2

            R = {}

            def diag(m1, m2, nm):
                a = add4(m1, m2)
                b = mul4(a, u)
                d = TT4()
                vec.tensor_scalar(out=d[:], in0=b[:], scalar1=-1.0,
                                  scalar2=1.0, op0=OP.mult, op1=OP.add)
                R[nm] = d

            def offd(m1, m2, op, nm):
                a = TT4()
                vec.tensor_tensor(out=a[:], in0=m1[:], in1=m2[:], op=op)
                R[nm] = mul4(a, u)

            diag(yy, zz, "R00"); diag(xx, zz, "R11"); diag(xx, yy, "R22")
            offd(xy, wz, OP.subtract, "R01"); offd(xz, wy, OP.add, "R02")
            offd(xy, wz, OP.add, "R10"); offd(yz, wx, OP.subtract, "R12")
            offd(xz, wy, OP.subtract, "R20"); offd(yz, wx, OP.add, "R21")

            # ---- obs streams in: p-planes slot-major first, tgt/w later ----
            obs = [pp.tile([P, FDT], f16, name=f"obs{k}") for k in range(7)]
            px, py, pz, X, Y, Z, W = obs
            for k in range(7):
                nc.sync.dma_start(obs[k][:], obs_d[k])

            # ---- rot: r = R*p + t per slot ----
            def rowplane():
                return pp.tile([P, FDT], f16, name=f"rp{tcnt[0]}")

            def rot_row(Rn0, Rn1, Rn2, tbase, nm):
                r = pp.tile([P, FDT], f16, name=nm)
                off = 0
                for s_, fd in enumerate(fds):
                    sl = slice(off, off + fd)
                    a = pp.tile([P, fd], f16, name=f"a{nm}{s_}")
                    vec.tensor_scalar(out=a[:], in0=pz[:, sl],
                                      scalar1=R[Rn2][:, s_:s_ + 1],
                                      scalar2=tabt[:, tbase + s_:tbase + s_ + 1],
                                      op0=OP.mult, op1=OP.add)
                    b = pp.tile([P, fd], f16, name=f"b{nm}{s_}")
                    vec.scalar_tensor_tensor(out=b[:], in0=py[:, sl],
                                             scalar=R[Rn1][:, s_:s_ + 1],
                                             in1=a[:], op0=OP.mult, op1=OP.add)
                    vec.scalar_tensor_tensor(out=r[:, sl], in0=px[:, sl],
                                             scalar=R[Rn0][:, s_:s_ + 1],
                                             in1=b[:], op0=OP.mult, op1=OP.add)
                    off += fd
                return r

            rx = rot_row("R00", "R01", "R02", 16, "rx")
            ry = rot_row("R10", "R11", "R12", 20, "ry")
            rz = rot_row("R20", "R21", "R22", 24, "rz")

            # ---- polar + residual in 2 column-chunks (slots 01 | 23) ----
            # az = arctan(ry/|rx|)*sign(rx) + pi*[rx<0]*sign(ry)
            # ACT emission order is phase-merged (ARS A, ARS B, trig A+B)
            # so the activation-table set loads only 3 times total.
            chunks = [(0, fds[0] + fds[1]), (fds[0] + fds[1], fds[2] + fds[3])]

            def T(cw):
                tcnt[0] += 1
                return pp.tile([P, cw], f16, name=f"pl{tcnt[0]}")

            def tt(cw, a, b, op, eng=vec):
                d = T(cw)
                eng.tensor_tensor(out=d[:], in0=a, in1=b, op=op)
                return d

            def afn(cw, a, fn):
                d = T(cw)
                act.activation(d[:], a, fn)
                return d

            st = []
            for ci, (c0, cw) in enumerate(chunks):
                cs = slice(c0, c0 + cw)
                rxc, ryc, rzc = rx[:, cs], ry[:, cs], rz[:, cs]
                # ACT set 1: square + abs_reciprocal_sqrt (one table set)
                sqx = afn(cw, rxc, AF.Square)
                sqy = afn(cw, ryc, AF.Square)
                sqz = tt(cw, rzc, rzc, OP.mult)
                rho2 = tt(cw, sqx[:], sqy[:], OP.add)
                r2 = tt(cw, rho2[:], sqz[:], OP.add)
                u2 = afn(cw, rho2[:], AF.Abs_reciprocal_sqrt)   # 1/rho
                ux = afn(cw, rxc, AF.Abs_reciprocal_sqrt)       # 1/sqrt|rx|
                ux2 = afn(cw, ux[:], AF.Square)                 # 1/|rx|
                st.append((cs, cw, rxc, ryc, rzc, r2, u2, ux2))

            # ACT set 2: sqrt -> range
            rngs = [afn(s_[1], s_[5][:], AF.Sqrt) for s_ in st]

            for ci, (cs, cw, rxc, ryc, rzc, r2, u2, ux2) in enumerate(st):
                last = ci == len(st) - 1
                rng = rngs[ci]
                e1 = tt(cw, rzc, u2[:], OP.mult)
                t0 = tt(cw, ryc, ux2[:], OP.mult)               # ry/|rx|
                # ACT set 2: arctan + sign
                el = afn(cw, e1[:], AF.Arctan)
                azp = afn(cw, t0[:], AF.Arctan)
                sgy = afn(cw, ryc, AF.Sign)
                sgx = afn(cw, rxc, AF.Sign)
                m = T(cw)
                vec.tensor_scalar(out=m[:], in0=rxc, scalar1=0.0, scalar2=PI,
                                  op0=OP.is_lt, op1=OP.mult)
                az1 = tt(cw, azp[:], sgx[:], OP.mult)
                corr = tt(cw, m[:], sgy[:], OP.mult, eng=gp)
                az = tt(cw, az1[:], corr[:], OP.add)

                for i, (pcomp, tgt) in enumerate(
                        ((rng, X[:, cs]), (az, Y[:, cs]), (el, Z[:, cs]))):
                    d = tt(cw, pcomp[:], tgt, OP.subtract)
                    o = tt(cw, d[:], W[:, cs], OP.mult,
                           eng=gp if (not last and i != 1) else vec)
                    nc.sync.dma_start(out_d[i, :, cs], o[:])
    nc.finalize()
    return nc


def _stage(poses, patch_coords, elevation_angle, poses_idx, patch_idx,
           target_coords, weights):
    """Pure-indexing host staging.  Returns (fds, tab array, obs array,
    gather indices for unpadding)."""
    poses = np.asarray(poses, dtype=np.float32)
    patch_coords = np.asarray(patch_coords, dtype=np.float32)
    elevation_angle = np.asarray(elevation_angle, dtype=np.float32)
    target_coords = np.asarray(target_coords, dtype=np.float32)
    weights = np.asarray(weights, dtype=np.float32)
    pid = np.asarray(poses_idx).astype(np.int64)
    qid = np.asarray(patch_idx).astype(np.int64)

    cnt = np.bincount(pid, minlength=NPOSE)
    order = np.argsort(-cnt, kind="stable")          # poses by count desc
    rank = np.empty(NPOSE, np.int64)
    rank[order] = np.arange(NPOSE)
    # tile k = ranks [128k,128(k+1)); tile -> core k%8, slot k//8
    # slot FD = count of first pose of tile 8s (max across cores)
    fds = tuple((int(cnt[order[128 * (8 * s)]]) + 7) & ~7 for s in range(NSLOT))
    FDT = int(sum(fds))
    slot_off = np.concatenate([[0], np.cumsum(fds)]).astype(np.int64)

    obs_rank = rank[pid]
    o = np.argsort(obs_rank, kind="stable")          # obs sorted by pose rank
    sorted_rank = obs_rank[o]
    start = np.searchsorted(sorted_rank, np.arange(NPOSE))
    within = np.arange(M, dtype=np.int64) - start[sorted_rank]

    k = sorted_rank >> 7                             # tile index
    part = sorted_rank & 127
    core = k & 7
    slot = k >> 3
    col = slot_off[slot] + within
    # flat position inside [NCORES, P, FDT]
    flat = (core * P + part) * FDT + col

    obs7 = np.zeros((7, NCORES * P * FDT), np.float16)
    obs7[0, flat] = patch_coords[qid[o], 0]
    obs7[1, flat] = patch_coords[qid[o], 1]
    obs7[2, flat] = elevation_angle[qid[o], 0]
    obs7[3, flat] = target_coords[o, 0]
    obs7[4, flat] = target_coords[o, 1]
    obs7[5, flat] = target_coords[o, 2]
    obs7[6, flat] = weights[o, 0]
    obs7 = obs7.reshape(7, NCORES, P, FDT).transpose(1, 0, 2, 3)
    obs7 = np.ascontiguousarray(obs7)

    # pose table per (core, slot, partition)
    pose_of = order.reshape(32, P)                   # [tile, partition]
    tab = np.zeros((NCORES, P, 7 * NSLOT), np.float32)
    for s in range(NSLOT):
        for c in range(NCORES):
            pidx = pose_of[8 * s + c]                # [128]
            for k in range(4):
                tab[c, :, 4 * k + s] = poses[pidx, 3 + k]    # qx..qw
            for k in range(3):
                tab[c, :, 16 + 4 * k + s] = poses[pidx, k]   # tx..tz
    return fds, tab, obs7, o, flat


def kernel(poses, patch_coords, elevation_angle, poses_idx, patch_idx,
           target_coords, weights):
    from concourse.bass_utils import run_bass_kernel_spmd

    fds, tab, obs7, o, flat = _stage(poses, patch_coords, elevation_angle,
                                     poses_idx, patch_idx, target_coords,
                                     weights)
    key = ("main", fds)
    if key not in _CACHE:
        _CACHE[key] = _build_main(fds)
    nc = _CACHE[key]
    res = run_bass_kernel_spmd(
        nc, [{"tab": tab[c], "obs": obs7[c]} for c in range(NCORES)],
        list(range(NCORES)))
    FDT = int(sum(fds))
    outp = np.stack([np.asarray(res.results[c]["out"])
                     for c in range(NCORES)])       # [8, 3, 128, FDT]
    outp = outp.transpose(0, 2, 3, 1).reshape(NCORES * P * FDT, 3)
    res_sorted = outp[flat]                          # [M, 3] in sorted order
    full = np.empty((M, 3), np.float32)
    full[o] = res_sorted.astype(np.float32)
    return full


# revision 8
# speedup vs baseline: 2.4688x; 1.0035x over previous
"""Trainium2 Bass kernel for nn_BoundleAdjustment (2M observations).

Single launch on all 8 NeuronCores.  Host work is pure indexing: observations
are sorted by pose and laid out pose-major, so each SBUF partition holds one
pose's observations per tile-slot and the per-pose rotation/translation become
[P,1] per-partition scalar operands — the 12 R/t streams of the old design
vanish from DMA and from the vector engine.

Device program (per core):
  1. tiny pose-table prologue: q -> R (f32, [128, 4] planes, ~27 ops)
  2. rot per slot: r = R*p + t via tensor_scalar + 2x scalar_tensor_tensor
     with [P,1] f32 scalars, fp16 streams
  3. planar polar+residual in fp16: range/az/el via Abs_reciprocal_sqrt and
     Arctan on the Scalar engine (2 activation-table sets only), quadrant
     fixup with tensor_scalar masks, weighted residual; a few off-critical
     tensor_tensor ops run on GPSIMD to unload the DVE.

Streams are fp16 (2x DVE mode, half the HBM bytes); per-pose scalars stay
f32, which keeps atan2 branch flips rare (rel RMS ~7e-3 vs fp32 reference).
"""

import numpy as np

M = 2097152
NCORES = 8
P = 128
NPOSE = 4096
NSLOT = 4          # tile slots per core; 8 cores * 4 slots * 128 poses = 4096
PI = float(np.pi)

_CACHE = {}


def _build_main(fds):
    import concourse.bass as bass
    import concourse.tile as tile
    from concourse import bacc, mybir

    nc = bacc.Bacc("TRN2", target_bir_lowering=False, debug=False,
                   num_devices=NCORES)
    f32 = mybir.dt.float32
    f16 = mybir.dt.float16
    OP = mybir.AluOpType
    AF = mybir.ActivationFunctionType
    FDT = int(sum(fds))

    tab_d = nc.declare_dram_parameter("tab", [P, 7 * NSLOT], f32,
                                      isOutput=False)
    obs_d = nc.declare_dram_parameter("obs", [7, P, FDT], f16, isOutput=False)
    out_d = nc.declare_dram_parameter("out", [3, P, FDT], f16, isOutput=True)

    with tile.TileContext(nc) as tc:
        with tc.tile_pool(name="pp", bufs=1) as pp:
            vec, act, gp = nc.vector, nc.scalar, nc.gpsimd

            # ---- pose table prologue (f32, FD=4; one [P,28] DMA) ----
            tabt = pp.tile([P, 7 * NSLOT], f32, name="tabt")
            nc.sync.dma_start(tabt[:], tab_d[:, :])
            qx, qy, qz, qw, tx, ty, tz = [
                tabt[:, 4 * k:4 * k + 4] for k in range(7)]

            tcnt = [0]

            def TT4():
                tcnt[0] += 1
                return pp.tile([P, NSLOT], f32, name=f"tt{tcnt[0]}")

            def _ap(x):
                return x if type(x).__name__ == "AP" else x[:]

            def mul4(a, b):
                d = TT4()
                vec.tensor_tensor(out=d[:], in0=_ap(a), in1=_ap(b), op=OP.mult)
                return d

            def add4(a, b):
                d = TT4()
                vec.tensor_tensor(out=d[:], in0=_ap(a), in1=_ap(b), op=OP.add)
                return d

            xx = mul4(qx, qx); yy = mul4(qy, qy)
            zz = mul4(qz, qz); ww = mul4(qw, qw)
            xy = mul4(qx, qy); xz = mul4(qx, qz); yz = mul4(qy, qz)
            wx = mul4(qw, qx); wy = mul4(qw, qy); wz = mul4(qw, qz)
            s = add4(add4(xx, yy), add4(zz, ww))
            half = TT4()
            vec.tensor_scalar(out=half[:], in0=s[:], scalar1=0.5, scalar2=None,
                              op0=OP.mult)
            u = TT4()
            vec.reciprocal(u[:], half[:])        # 2/|q|^2

            R = {}

            def diag(m1, m2, nm):
                a = add4(m1, m2)
                b = mul4(a, u)
                d = TT4()
                vec.tensor_scalar(out=d[:], in0=b[:], scalar1=-1.0,
                                  scalar2=1.0, op0=OP.mult, op1=OP.add)
                R[nm] = d

            def offd(m1, m2, op, nm):
                a = TT4()
                vec.tensor_tensor(out=a[:], in0=m1[:], in1=m2[:], op=op)
                R[nm] = mul4(a, u)

            diag(yy, zz, "R00"); diag(xx, zz, "R11"); diag(xx, yy, "R22")
            offd(xy, wz, OP.subtract, "R01"); offd(xz, wy, OP.add, "R02")
            offd(xy, wz, OP.add, "R10"); offd(yz, wx, OP.subtract, "R12")
            offd(xz, wy, OP.subtract, "R20"); offd(yz, wx, OP.add, "R21")

            # ---- obs streams in: p-planes slot-major first, tgt/w later ----
            obs = [pp.tile([P, FDT], f16, name=f"obs{k}") for k in range(7)]
            px, py, pz, X, Y, Z, W = obs
            half = fds[0] + fds[1]
            for k in (2, 1, 0):      # pz first: rot's tensor_scalar reads it
                nc.sync.dma_start(obs[k][:, 0:half], obs_d[k, :, 0:half])
            for k in (2, 1, 0):
                nc.sync.dma_start(obs[k][:, half:FDT], obs_d[k, :, half:FDT])
            for k in (3, 4, 5, 6):
                nc.sync.dma_start(obs[k][:], obs_d[k])

            # ---- rot: r = R*p + t per slot ----
            def rowplane():
                return pp.tile([P, FDT], f16, name=f"rp{tcnt[0]}")

            def rot_row(Rn0, Rn1, Rn2, tbase, nm):
                r = pp.tile([P, FDT], f16, name=nm)
                off = 0
                for s_, fd in enumerate(fds):
                    sl = slice(off, off + fd)
                    a = pp.tile([P, fd], f16, name=f"a{nm}{s_}")
                    vec.tensor_scalar(out=a[:], in0=pz[:, sl],
                                      scalar1=R[Rn2][:, s_:s_ + 1],
                                      scalar2=tabt[:, tbase + s_:tbase + s_ + 1],
                                      op0=OP.mult, op1=OP.add)
                    b = pp.tile([P, fd], f16, name=f"b{nm}{s_}")
                    vec.scalar_tensor_tensor(out=b[:], in0=py[:, sl],
                                             scalar=R[Rn1][:, s_:s_ + 1],
                                             in1=a[:], op0=OP.mult, op1=OP.add)
                    vec.scalar_tensor_tensor(out=r[:, sl], in0=px[:, sl],
                                             scalar=R[Rn0][:, s_:s_ + 1],
                                             in1=b[:], op0=OP.mult, op1=OP.add)
                    off += fd
                return r

            rx = rot_row("R00", "R01", "R02", 16, "rx")
            ry = rot_row("R10", "R11", "R12", 20, "ry")
            rz = rot_row("R20", "R21", "R22", 24, "rz")

            # ---- polar + residual in 2 column-chunks (slots 01 | 23) ----
            # az = arctan(ry/|rx|)*sign(rx) + pi*[rx<0]*sign(ry)
            # ACT emission order is phase-merged (ARS A, ARS B, trig A+B)
            # so the activation-table set loads only 3 times total.
            chunks = [(0, fds[0] + fds[1]), (fds[0] + fds[1], fds[2] + fds[3])]

            def T(cw):
                tcnt[0] += 1
                return pp.tile([P, cw], f16, name=f"pl{tcnt[0]}")

            def tt(cw, a, b, op, eng=vec):
                d = T(cw)
                eng.tensor_tensor(out=d[:], in0=a, in1=b, op=op)
                return d

            def afn(cw, a, fn):
                d = T(cw)
                act.activation(d[:], a, fn)
                return d

            st = []
            for ci, (c0, cw) in enumerate(chunks):
                cs = slice(c0, c0 + cw)
                rxc, ryc, rzc = rx[:, cs], ry[:, cs], rz[:, cs]
                # ACT set 1: square + abs_reciprocal_sqrt (one table set)
                sqx = afn(cw, rxc, AF.Square)
                sqy = afn(cw, ryc, AF.Square)
                sqz = afn(cw, rzc, AF.Square)
                rho2 = tt(cw, sqx[:], sqy[:], OP.add)
                r2 = tt(cw, rho2[:], sqz[:], OP.add)
                u2 = afn(cw, rho2[:], AF.Abs_reciprocal_sqrt)   # 1/rho
                ux = afn(cw, rxc, AF.Abs_reciprocal_sqrt)       # 1/sqrt|rx|
                ux2 = afn(cw, ux[:], AF.Square)                 # 1/|rx|
                st.append((cs, cw, rxc, ryc, rzc, r2, u2, ux2))

            # ACT set 2: sqrt -> range
            rngs = [afn(s_[1], s_[5][:], AF.Sqrt) for s_ in st]

            for ci, (cs, cw, rxc, ryc, rzc, r2, u2, ux2) in enumerate(st):
                last = ci == len(st) - 1
                rng = rngs[ci]
                e1 = tt(cw, rzc, u2[:], OP.mult)
                t0 = tt(cw, ryc, ux2[:], OP.mult)               # ry/|rx|
                # ACT set 2: arctan + sign
                el = afn(cw, e1[:], AF.Arctan)
                azp = afn(cw, t0[:], AF.Arctan)
                sgy = afn(cw, ryc, AF.Sign)
                sgx = afn(cw, rxc, AF.Sign)
                m = T(cw)
                vec.tensor_scalar(out=m[:], in0=rxc, scalar1=0.0, scalar2=PI,
                                  op0=OP.is_lt, op1=OP.mult)
                az1 = tt(cw, azp[:], sgx[:], OP.mult)
                corr = tt(cw, m[:], sgy[:], OP.mult, eng=gp)
                az = tt(cw, az1[:], corr[:], OP.add)

                for i, (pcomp, tgt) in enumerate(
                        ((rng, X[:, cs]), (az, Y[:, cs]), (el, Z[:, cs]))):
                    d = tt(cw, pcomp[:], tgt, OP.subtract)
                    o = tt(cw, d[:], W[:, cs], OP.mult,
                           eng=gp if (not last and i != 1) else vec)
                    nc.sync.dma_start(out_d[i, :, cs], o[:])
    nc.finalize()
    return nc


def _stage(poses, patch_coords, elevation_angle, poses_idx, patch_idx,
           target_coords, weights):
    """Pure-indexing host staging.  Returns (fds, tab array, obs array,
    gather indices for unpadding)."""
    poses = np.asarray(poses, dtype=np.float32)
    patch_coords = np.asarray(patch_coords, dtype=np.float32)
    elevation_angle = np.asarray(elevation_angle, dtype=np.float32)
    target_coords = np.asarray(target_coords, dtype=np.float32)
    weights = np.asarray(weights, dtype=np.float32)
    pid = np.asarray(poses_idx).astype(np.int64)
    qid = np.asarray(patch_idx).astype(np.int64)

    cnt = np.bincount(pid, minlength=NPOSE)
    order = np.argsort(-cnt, kind="stable")          # poses by count desc
    rank = np.empty(NPOSE, np.int64)
    rank[order] = np.arange(NPOSE)
    # tile k = ranks [128k,128(k+1)); tile -> core k%8, slot k//8
    # slot FD = count of first pose of tile 8s (max across cores)
    fds = tuple((int(cnt[order[128 * (8 * s)]]) + 7) & ~7 for s in range(NSLOT))
    FDT = int(sum(fds))
    slot_off = np.concatenate([[0], np.cumsum(fds)]).astype(np.int64)

    obs_rank = rank[pid]
    o = np.argsort(obs_rank, kind="stable")          # obs sorted by pose rank
    sorted_rank = obs_rank[o]
    start = np.searchsorted(sorted_rank, np.arange(NPOSE))
    within = np.arange(M, dtype=np.int64) - start[sorted_rank]

    k = sorted_rank >> 7                             # tile index
    part = sorted_rank & 127
    core = k & 7
    slot = k >> 3
    col = slot_off[slot] + within
    # flat position inside [NCORES, P, FDT]
    flat = (core * P + part) * FDT + col

    obs7 = np.zeros((7, NCORES * P * FDT), np.float16)
    obs7[0, flat] = patch_coords[qid[o], 0]
    obs7[1, flat] = patch_coords[qid[o], 1]
    obs7[2, flat] = elevation_angle[qid[o], 0]
    obs7[3, flat] = target_coords[o, 0]
    obs7[4, flat] = target_coords[o, 1]
    obs7[5, flat] = target_coords[o, 2]
    obs7[6, flat] = weights[o, 0]
    obs7 = obs7.reshape(7, NCORES, P, FDT).transpose(1, 0, 2, 3)
    obs7 = np.ascontiguousarray(obs7)

    # pose table per (core, slot, partition)
    pose_of = order.reshape(32, P)                   # [tile, partition]
    tab = np.zeros((NCORES, P, 7 * NSLOT), np.float32)
    for s in range(NSLOT):
        for c in range(NCORES):
            pidx = pose_of[8 * s + c]                # [128]
            for k in range(4):
                tab[c, :, 4 * k + s] = poses[pidx, 3 + k]    # qx..qw
            for k in range(3):
                tab[c, :, 16 + 4 * k + s] = poses[pidx, k]   # tx..tz
    return fds, tab, obs7, o, flat


def kernel(poses, patch_coords, elevation_angle, poses_idx, patch_idx,
           target_coords, weights):
    from concourse.bass_utils import run_bass_kernel_spmd

    fds, tab, obs7, o, flat = _stage(poses, patch_coords, elevation_angle,
                                     poses_idx, patch_idx, target_coords,
                                     weights)
    key = ("main", fds)
    if key not in _CACHE:
        _CACHE[key] = _build_main(fds)
    nc = _CACHE[key]
    res = run_bass_kernel_spmd(
        nc, [{"tab": tab[c], "obs": obs7[c]} for c in range(NCORES)],
        list(range(NCORES)))
    FDT = int(sum(fds))
    outp = np.stack([np.asarray(res.results[c]["out"])
                     for c in range(NCORES)])       # [8, 3, 128, FDT]
    outp = outp.transpose(0, 2, 3, 1).reshape(NCORES * P * FDT, 3)
    res_sorted = outp[flat]                          # [M, 3] in sorted order
    full = np.empty((M, 3), np.float32)
    full[o] = res_sorted.astype(np.float32)
    return full
